# revision 6
# baseline (speedup 1.0000x reference)
"""Trainium2 Bass kernel for nn_CMix_x060moe (RWKV CMix + hash-routed MoE).

Strategy: expert-sharded SPMD over 8 NeuronCores. Hash routing depends only
on token_ids, so the host computes the token->expert assignment as part of
sharding: core e receives exactly 2048 tokens (expert e's kept tokens in
FIFO order, padded with capacity-dropped tokens from anywhere, mask=0 for
those). The host also performs the token shift (xk/xr mixing) since it is
pure data movement; each core computes the dense squared-ReLU FFN, its own
expert's FFN and the sigmoid receptance for its 2048 tokens; the host
scatters rows back. No collectives needed and the load is perfectly
balanced.

All activations live C-major ([C, tokens]) on device so every matmul keeps
weights as the stationary operand. Matmuls run in bf16 (activations and
weights), which enables the PE fast-weight-load path, halves HBM traffic
and doubles DVE throughput; PSUM accumulation stays fp32, and the
dense/expert outputs are combined in fp32.
"""

import os

import ml_dtypes
import numpy as np

import concourse.mybir as mybir
import concourse.tile as tile
from concourse import bacc
from concourse.bass_utils import run_bass_kernel_spmd

LAST_RESULTS = None  # set on every kernel() call; holds BassKernelResults

B, T, C = 8, 2048, 1024
DFF, DFFE = 4096, 2048
E = 8
HASH_PRIME = 5099
CAP = (B * T) // E  # 2048
N = B * T

P = 128               # partitions
TB = 512              # matmul token width (psum bank)
NH = CAP // TB        # 4 token chunks
CT = C // P           # 8  C-tiles
MT_D = DFF // P       # 32 dense-hidden tiles
MT_E = DFFE // P      # 16 expert-hidden tiles
GD = 4                # dense second-layer contraction groups
GE = 2                # expert second-layer contraction groups
HD = MT_D // GD       # 8 k-tiles per dense group
HE = MT_E // GE       # 8 k-tiles per expert group

F32 = mybir.dt.float32
BF16 = mybir.dt.bfloat16

_COMPILED = None


def _build():
    nc = bacc.Bacc(trn_type="TRN2")

    xk = nc.dram_tensor("xk", [CT, P, CAP], BF16, kind="ExternalInput")
    xr = nc.dram_tensor("xr", [CT, P, CAP], BF16, kind="ExternalInput")
    maskd = nc.dram_tensor("maskd", [P, CAP], BF16, kind="ExternalInput")
    # weights, host-tiled p-major: w*[m][p][k*P+q] = W[k*P+p, m*P+q]
    wk = nc.dram_tensor("wk", [MT_D, P, CT * P], BF16, kind="ExternalInput")
    wv = nc.dram_tensor("wv", [CT, P, MT_D * P], BF16, kind="ExternalInput")
    wr = nc.dram_tensor("wr", [CT, P, CT * P], BF16, kind="ExternalInput")
    wek = nc.dram_tensor("wek", [MT_E, P, CT * P], BF16, kind="ExternalInput")
    wev = nc.dram_tensor("wev", [CT, P, MT_E * P], BF16, kind="ExternalInput")
    yout = nc.dram_tensor("y", [CT, P, CAP], F32, kind="ExternalOutput")

    with tile.TileContext(nc) as tc:
        with (
            tc.tile_pool(name="const", bufs=1) as constp,
            tc.tile_pool(name="acts", bufs=1) as acts,
            tc.tile_pool(name="wfirst", bufs=3) as wfp,
            tc.tile_pool(name="wsecond", bufs=2) as wsp,
            tc.tile_pool(name="tmp", bufs=2) as tmpp,
            tc.tile_pool(name="outp", bufs=2) as outp,
            tc.tile_pool(name="ps1", bufs=3, space="PSUM") as ps1,
            tc.tile_pool(name="ps2", bufs=2, space="PSUM") as ps2,
            tc.tile_pool(name="psr", bufs=3, space="PSUM") as psr,
        ):
            chunks = [slice(h * TB, (h + 1) * TB) for h in range(NH)]

            # activations (full residency, host-precomputed token shift).
            # DMA issue order matters: the sync engine posts descriptors
            # serially (~0.6us each), so only chunk 0 of xk goes ahead of
            # the first weight tile; the rest follows as one big transfer
            # per tile, and xr/mask wait until the expert phase.
            xkt = [acts.tile([P, CAP], BF16, tag=f"xk{i}", name=f"xk{i}")
                   for i in range(CT)]
            xrt = [acts.tile([P, CAP], BF16, tag=f"xr{i}", name=f"xr{i}")
                   for i in range(CT)]
            # first dense weight tile goes ahead of everything
            wt_first = wfp.tile([P, CT * P], BF16, tag="wk")
            nc.sync.dma_start(wt_first[:], wk[0])
            for ct in range(CT):
                nc.sync.dma_start(xkt[ct][:, chunks[0]], xk[ct, :, chunks[0]])

            kv = [acts.tile([P, CAP], F32, tag=f"kv{i}", name=f"kv{i}")
                  for i in range(CT)]
            # dense/expert hidden tiles share one set of buffers
            kt = [acts.tile([P, CAP], BF16, tag=f"kt{i}", name=f"kt{i}")
                  for i in range(HD)]

            # ---- dense: k = relu(xk@Wk)^2 ; kv = k @ Wv  (grouped) ----
            for g in range(GD):
                for i in range(HD):
                    m = g * HD + i
                    if m == 0:
                        wt = wt_first
                        rest = slice(TB, CAP)
                        for ct in range(CT):
                            nc.sync.dma_start(xkt[ct][:, rest], xk[ct, :, rest])
                    else:
                        wt = wfp.tile([P, CT * P], BF16, tag="wk")
                        nc.sync.dma_start(wt[:], wk[m])
                    for h in range(NH):
                        pd = ps1.tile([P, TB], F32, tag="ps1")
                        for k in range(CT):
                            nc.tensor.matmul(
                                pd[:], wt[:, k * P:(k + 1) * P],
                                xkt[k][:, chunks[h]],
                                start=(k == 0), stop=(k == CT - 1),
                            )
                        rl = tmpp.tile([P, TB], BF16, tag="rl", bufs=3)
                        nc.scalar.activation(
                            rl[:], pd[:], mybir.ActivationFunctionType.Relu
                        )
                        nc.vector.tensor_tensor(
                            out=kt[i][:, chunks[h]], in0=rl[:], in1=rl[:],
                            op=mybir.AluOpType.mult,
                        )
                for m in range(CT):
                    wt = wsp.tile([P, HD * P], BF16, tag="wv")
                    nc.sync.dma_start(
                        wt[:], wv[m, :, g * HD * P:(g + 1) * HD * P]
                    )
                    for h in range(NH):
                        pv = ps2.tile([P, TB], F32, tag="ps2")
                        for k in range(HD):
                            nc.tensor.matmul(
                                pv[:], wt[:, k * P:(k + 1) * P],
                                kt[k][:, chunks[h]],
                                start=(k == 0), stop=(k == HD - 1),
                            )
                        if g == 0:
                            nc.vector.tensor_copy(kv[m][:, chunks[h]], pv[:])
                        else:
                            nc.vector.tensor_tensor(
                                out=kv[m][:, chunks[h]], in0=pv[:],
                                in1=kv[m][:, chunks[h]],
                                op=mybir.AluOpType.add,
                            )

            tmask = constp.tile([P, CAP], BF16)
            nc.sync.dma_start(tmask[:], maskd[:])
            for ct in range(CT):
                nc.sync.dma_start(xrt[ct][:], xr[ct])

            # ---- expert: kv += mask * (relu(xk@Wek)^2 @ Wev) (grouped) ----
            for g in range(GE):
                for i in range(HE):
                    m = g * HE + i
                    wt = wfp.tile([P, CT * P], BF16, tag="wek")
                    nc.sync.dma_start(wt[:], wek[m])
                    for h in range(NH):
                        pd = ps1.tile([P, TB], F32, tag="ps1")
                        for k in range(CT):
                            nc.tensor.matmul(
                                pd[:], wt[:, k * P:(k + 1) * P],
                                xkt[k][:, chunks[h]],
                                start=(k == 0), stop=(k == CT - 1),
                            )
                        rl = tmpp.tile([P, TB], BF16, tag="rl", bufs=3)
                        nc.scalar.activation(
                            rl[:], pd[:], mybir.ActivationFunctionType.Relu
                        )
                        nc.vector.tensor_tensor(
                            out=kt[i][:, chunks[h]], in0=rl[:], in1=rl[:],
                            op=mybir.AluOpType.mult,
                        )
                for m in range(CT):
                    wt = wsp.tile([P, HE * P], BF16, tag="wev")
                    nc.sync.dma_start(
                        wt[:], wev[m, :, g * HE * P:(g + 1) * HE * P]
                    )
                    for h in range(NH):
                        po = ps2.tile([P, TB], F32, tag="ps2")
                        for k in range(HE):
                            nc.tensor.matmul(
                                po[:], wt[:, k * P:(k + 1) * P],
                                kt[k][:, chunks[h]],
                                start=(k == 0), stop=(k == HE - 1),
                            )
                        # vector drains PSUM (gpsimd can't read PSUM);
                        # the kv accumulate runs on the idle gpsimd engine
                        cm = tmpp.tile([P, TB], BF16, tag="cmb", bufs=2)
                        nc.vector.tensor_tensor(
                            out=cm[:], in0=po[:], in1=tmask[:, chunks[h]],
                            op=mybir.AluOpType.mult,
                        )
                        nc.gpsimd.tensor_tensor(
                            out=kv[m][:, chunks[h]], in0=cm[:],
                            in1=kv[m][:, chunks[h]],
                            op=mybir.AluOpType.add,
                        )

            # ---- receptance last: y = sigmoid(xr @ Wr) * kv ----
            for m in range(CT):
                wt = wfp.tile([P, CT * P], BF16, tag="wr")
                nc.sync.dma_start(wt[:], wr[m])
                for h in range(NH):
                    pr = psr.tile([P, TB], F32, tag="psr")
                    for k in range(CT):
                        nc.tensor.matmul(
                            pr[:], wt[:, k * P:(k + 1) * P],
                            xrt[k][:, chunks[h]],
                            start=(k == 0), stop=(k == CT - 1),
                        )
                    rm = tmpp.tile([P, TB], BF16, tag="rm", bufs=3)
                    nc.scalar.activation(
                        rm[:], pr[:], mybir.ActivationFunctionType.Sigmoid
                    )
                    yo = outp.tile([P, TB], F32, tag="yo")
                    nc.vector.tensor_tensor(
                        out=yo[:], in0=kv[m][:, chunks[h]], in1=rm[:],
                        op=mybir.AluOpType.mult,
                    )
                    nc.sync.dma_start(yout[m, :, chunks[h]], yo[:])

    nc.compile()
    return nc


def _routing(token_ids: np.ndarray):
    """Token -> (per-core global token list [E, CAP], per-core keep mask)."""
    tid = token_ids.reshape(N).astype(np.int64)
    eidx = (tid * HASH_PRIME) % E
    order = np.argsort(eidx, kind="stable")  # FIFO within expert
    counts = np.bincount(eidx, minlength=E)
    starts = np.zeros(E + 1, np.int64)
    np.cumsum(counts, out=starts[1:])

    token_lists = np.empty((E, CAP), np.int64)
    masks = np.zeros((E, CAP), np.float32)
    dropped = []
    fill_needed = []
    for e in range(E):
        grp = order[starts[e]:starts[e + 1]]
        nk = min(len(grp), CAP)
        token_lists[e, :nk] = grp[:nk]
        masks[e, :nk] = 1.0
        dropped.append(grp[CAP:])
        fill_needed.append(CAP - nk)
    dropped = (
        np.concatenate(dropped) if dropped else np.empty(0, np.int64)
    )
    pos = 0
    for e in range(E):
        need = fill_needed[e]
        if need:
            token_lists[e, CAP - need:] = dropped[pos:pos + need]
            pos += need
    assert pos == len(dropped)
    return token_lists, masks


def _tile_first(W, mt):
    """[C, M] -> [mt, P, CT*P] with w[m][p][k*P+q] = W[k*P+p, m*P+q]."""
    ct = W.shape[0] // P
    return np.ascontiguousarray(
        W.reshape(ct, P, mt, P).transpose(2, 1, 0, 3).reshape(mt, P, ct * P)
    ).astype(ml_dtypes.bfloat16)


def _tile_second(W, ct_out):
    """[K, M] -> [ct_out, P, KT*P] with w[m][p][k*P+q] = W[k*P+p, m*P+q]."""
    kt = W.shape[0] // P
    return np.ascontiguousarray(
        W.reshape(kt, P, ct_out, P).transpose(2, 1, 0, 3).reshape(ct_out, P, kt * P)
    ).astype(ml_dtypes.bfloat16)


def kernel(x, shift_state, token_ids, time_maa_k, time_maa_r, Wk, Wv, Wr, Wek, Wev):
    global _COMPILED
    if _COMPILED is None:
        _COMPILED = _build()
    nc = _COMPILED

    x = np.asarray(x, np.float32)
    shift_state = np.asarray(shift_state, np.float32)
    token_lists, masks = _routing(np.asarray(token_ids))

    # token shift on host (pure data movement + broadcast mix)
    xf = x.reshape(N, C)
    xprev_f = np.empty_like(xf)
    xprev_f[1:] = xf[:-1]
    xprev_f[np.arange(B) * T] = shift_state
    maak = np.asarray(time_maa_k, np.float32)
    maar = np.asarray(time_maa_r, np.float32)
    dx = xprev_f - xf
    xk_f = (xf + dx * maak).astype(ml_dtypes.bfloat16)
    xr_f = (xf + dx * maar).astype(ml_dtypes.bfloat16)

    wk_t = _tile_first(np.asarray(Wk, np.float32), MT_D)
    wr_t = _tile_first(np.asarray(Wr, np.float32), CT)
    wv_t = _tile_second(np.asarray(Wv, np.float32), CT)
    Wek = np.asarray(Wek, np.float32)
    Wev = np.asarray(Wev, np.float32)

    def ctmajor(rows):  # [CAP, C] bf16 -> [CT, P, CAP]
        return np.ascontiguousarray(rows.T.reshape(CT, P, CAP))

    in_maps = []
    for e in range(E):
        L = token_lists[e]
        in_maps.append(dict(
            xk=ctmajor(xk_f[L]),
            xr=ctmajor(xr_f[L]),
            maskd=np.ascontiguousarray(
                np.broadcast_to(masks[e], (P, CAP))
            ).astype(ml_dtypes.bfloat16),
            wk=wk_t,
            wv=wv_t,
            wr=wr_t,
            wek=_tile_first(Wek[e], MT_E),
            wev=_tile_second(Wev[e], CT),
        ))

    res = run_bass_kernel_spmd(
        nc, in_maps, core_ids=list(range(E)),
        trace=bool(os.environ.get("KERNEL_TRACE")),
    )
    global LAST_RESULTS
    LAST_RESULTS = res

    y = np.empty((N, C), np.float32)
    for e in range(E):
        y[token_lists[e]] = res.results[e]["y"].reshape(C, CAP).T
    return y.reshape(B, T, C)


# revision 7
# speedup vs baseline: 1.0072x; 1.0072x over previous
"""Trainium2 Bass kernel for nn_CMix_x060moe (RWKV CMix + hash-routed MoE).

Strategy: expert-sharded SPMD over 8 NeuronCores. Hash routing depends only
on token_ids, so the host computes the token->expert assignment as part of
sharding: core e receives exactly 2048 tokens (expert e's kept tokens in
FIFO order, padded with capacity-dropped tokens from anywhere, mask=0 for
those). The host also performs the token shift (xk/xr mixing) since it is
pure data movement; each core computes the dense squared-ReLU FFN, its own
expert's FFN and the sigmoid receptance for its 2048 tokens; the host
scatters rows back. No collectives needed and the load is perfectly
balanced.

All activations live C-major ([C, tokens]) on device so every matmul keeps
weights as the stationary operand. Matmuls run in bf16 (activations and
weights), which enables the PE fast-weight-load path, halves HBM traffic
and doubles DVE throughput; PSUM accumulation stays fp32, and the
dense/expert outputs are combined in fp32.
"""

import os

import ml_dtypes
import numpy as np

import concourse.mybir as mybir
import concourse.tile as tile
from concourse import bacc
from concourse.bass_utils import run_bass_kernel_spmd

LAST_RESULTS = None  # set on every kernel() call; holds BassKernelResults

B, T, C = 8, 2048, 1024
DFF, DFFE = 4096, 2048
E = 8
HASH_PRIME = 5099
CAP = (B * T) // E  # 2048
N = B * T

P = 128               # partitions
TB = 512              # matmul token width (psum bank)
NH = CAP // TB        # 4 token chunks
CT = C // P           # 8  C-tiles
MT_D = DFF // P       # 32 dense-hidden tiles
MT_E = DFFE // P      # 16 expert-hidden tiles
GD = 4                # dense second-layer contraction groups
GE = 2                # expert second-layer contraction groups
HD = MT_D // GD       # 8 k-tiles per dense group
HE = MT_E // GE       # 8 k-tiles per expert group

F32 = mybir.dt.float32
BF16 = mybir.dt.bfloat16
FP8 = mybir.dt.float8e4
WR_SCALE = 128.0  # Wr is pre-scaled into fp8 normal range; sigmoid rescales

_COMPILED = None


def _build():
    nc = bacc.Bacc(trn_type="TRN2")

    xk = nc.dram_tensor("xk", [CT, P, CAP], BF16, kind="ExternalInput")
    xr = nc.dram_tensor("xr", [CT // 2, P, 2, CAP], FP8, kind="ExternalInput")
    maskd = nc.dram_tensor("maskd", [P, CAP], BF16, kind="ExternalInput")
    # weights, host-tiled p-major: w*[m][p][k*P+q] = W[k*P+p, m*P+q]
    wk = nc.dram_tensor("wk", [MT_D, P, CT * P], BF16, kind="ExternalInput")
    wv = nc.dram_tensor("wv", [CT, P, MT_D * P], BF16, kind="ExternalInput")
    wr = nc.dram_tensor("wr", [CT, P, CT // 2, 2, P], FP8, kind="ExternalInput")
    wek = nc.dram_tensor("wek", [MT_E, P, CT * P], BF16, kind="ExternalInput")
    wev = nc.dram_tensor("wev", [CT, P, MT_E * P], BF16, kind="ExternalInput")
    yout = nc.dram_tensor("y", [CT, P, CAP], F32, kind="ExternalOutput")

    with tile.TileContext(nc) as tc:
        with (
            tc.tile_pool(name="const", bufs=1) as constp,
            tc.tile_pool(name="acts", bufs=1) as acts,
            tc.tile_pool(name="wfirst", bufs=3) as wfp,
            tc.tile_pool(name="wsecond", bufs=2) as wsp,
            tc.tile_pool(name="tmp", bufs=2) as tmpp,
            tc.tile_pool(name="outp", bufs=2) as outp,
            tc.tile_pool(name="ps1", bufs=3, space="PSUM") as ps1,
            tc.tile_pool(name="ps2", bufs=2, space="PSUM") as ps2,
            tc.tile_pool(name="psr", bufs=3, space="PSUM") as psr,
        ):
            chunks = [slice(h * TB, (h + 1) * TB) for h in range(NH)]

            # activations (full residency, host-precomputed token shift).
            # DMA issue order matters: the sync engine posts descriptors
            # serially (~0.6us each), so only chunk 0 of xk goes ahead of
            # the first weight tile; the rest follows as one big transfer
            # per tile, and xr/mask wait until the expert phase.
            xkt = [acts.tile([P, CAP], BF16, tag=f"xk{i}", name=f"xk{i}")
                   for i in range(CT)]
            xrt = [acts.tile([P, 2, CAP], FP8, tag=f"xr{i}", name=f"xr{i}")
                   for i in range(CT // 2)]
            # first dense weight tile goes ahead of everything
            wt_first = wfp.tile([P, CT * P], BF16, tag="wk")
            nc.sync.dma_start(wt_first[:], wk[0])
            for ct in range(CT):
                nc.sync.dma_start(xkt[ct][:, chunks[0]], xk[ct, :, chunks[0]])

            kv = [acts.tile([P, CAP], F32, tag=f"kv{i}", name=f"kv{i}")
                  for i in range(CT)]
            # dense/expert hidden tiles share one set of buffers
            kt = [acts.tile([P, CAP], BF16, tag=f"kt{i}", name=f"kt{i}")
                  for i in range(HD)]

            # ---- dense: k = relu(xk@Wk)^2 ; kv = k @ Wv  (grouped) ----
            for g in range(GD):
                for i in range(HD):
                    m = g * HD + i
                    if m == 0:
                        wt = wt_first
                        rest = slice(TB, CAP)
                        for ct in range(CT):
                            nc.sync.dma_start(xkt[ct][:, rest], xk[ct, :, rest])
                    else:
                        wt = wfp.tile([P, CT * P], BF16, tag="wk")
                        nc.sync.dma_start(wt[:], wk[m])
                    for h in range(NH):
                        pd = ps1.tile([P, TB], F32, tag="ps1")
                        for k in range(CT):
                            nc.tensor.matmul(
                                pd[:], wt[:, k * P:(k + 1) * P],
                                xkt[k][:, chunks[h]],
                                start=(k == 0), stop=(k == CT - 1),
                            )
                        rl = tmpp.tile([P, TB], BF16, tag="rl", bufs=3)
                        nc.scalar.activation(
                            rl[:], pd[:], mybir.ActivationFunctionType.Relu
                        )
                        nc.vector.tensor_tensor(
                            out=kt[i][:, chunks[h]], in0=rl[:], in1=rl[:],
                            op=mybir.AluOpType.mult,
                        )
                for m in range(CT):
                    wt = wsp.tile([P, HD * P], BF16, tag="wv")
                    nc.sync.dma_start(
                        wt[:], wv[m, :, g * HD * P:(g + 1) * HD * P]
                    )
                    for h in range(NH):
                        pv = ps2.tile([P, TB], F32, tag="ps2")
                        for k in range(HD):
                            nc.tensor.matmul(
                                pv[:], wt[:, k * P:(k + 1) * P],
                                kt[k][:, chunks[h]],
                                start=(k == 0), stop=(k == HD - 1),
                            )
                        if g == 0:
                            nc.vector.tensor_copy(kv[m][:, chunks[h]], pv[:])
                        else:
                            nc.vector.tensor_tensor(
                                out=kv[m][:, chunks[h]], in0=pv[:],
                                in1=kv[m][:, chunks[h]],
                                op=mybir.AluOpType.add,
                            )

            tmask = constp.tile([P, CAP], BF16)
            nc.sync.dma_start(tmask[:], maskd[:])
            for j in range(CT // 2):
                nc.sync.dma_start(xrt[j][:], xr[j])

            # ---- expert: kv += mask * (relu(xk@Wek)^2 @ Wev) (grouped) ----
            for g in range(GE):
                for i in range(HE):
                    m = g * HE + i
                    wt = wfp.tile([P, CT * P], BF16, tag="wek")
                    nc.sync.dma_start(wt[:], wek[m])
                    for h in range(NH):
                        pd = ps1.tile([P, TB], F32, tag="ps1")
                        for k in range(CT):
                            nc.tensor.matmul(
                                pd[:], wt[:, k * P:(k + 1) * P],
                                xkt[k][:, chunks[h]],
                                start=(k == 0), stop=(k == CT - 1),
                            )
                        rl = tmpp.tile([P, TB], BF16, tag="rl", bufs=3)
                        nc.scalar.activation(
                            rl[:], pd[:], mybir.ActivationFunctionType.Relu
                        )
                        nc.vector.tensor_tensor(
                            out=kt[i][:, chunks[h]], in0=rl[:], in1=rl[:],
                            op=mybir.AluOpType.mult,
                        )
                for m in range(CT):
                    wt = wsp.tile([P, HE * P], BF16, tag="wev")
                    nc.sync.dma_start(
                        wt[:], wev[m, :, g * HE * P:(g + 1) * HE * P]
                    )
                    for h in range(NH):
                        po = ps2.tile([P, TB], F32, tag="ps2")
                        for k in range(HE):
                            nc.tensor.matmul(
                                po[:], wt[:, k * P:(k + 1) * P],
                                kt[k][:, chunks[h]],
                                start=(k == 0), stop=(k == HE - 1),
                            )
                        # vector drains PSUM (gpsimd can't read PSUM);
                        # the kv accumulate runs on the idle gpsimd engine
                        cm = tmpp.tile([P, TB], BF16, tag="cmb", bufs=2)
                        nc.vector.tensor_tensor(
                            out=cm[:], in0=po[:], in1=tmask[:, chunks[h]],
                            op=mybir.AluOpType.mult,
                        )
                        nc.gpsimd.tensor_tensor(
                            out=kv[m][:, chunks[h]], in0=cm[:],
                            in1=kv[m][:, chunks[h]],
                            op=mybir.AluOpType.add,
                        )

            # ---- receptance last: y = sigmoid(xr @ Wr) * kv ----
            for m in range(CT):
                wt = wfp.tile([P, CT // 2, 2, P], FP8, tag="wr")
                nc.sync.dma_start(wt[:], wr[m])
                for h in range(NH):
                    pr = psr.tile([P, TB], F32, tag="psr")
                    for j in range(CT // 2):
                        nc.tensor.matmul(
                            pr[:], wt[:, j], xrt[j][:, :, chunks[h]],
                            start=(j == 0), stop=(j == CT // 2 - 1),
                            perf_mode=mybir.MatmulPerfMode.DoubleRow,
                        )
                    rm = tmpp.tile([P, TB], BF16, tag="rm", bufs=3)
                    nc.scalar.activation(
                        rm[:], pr[:], mybir.ActivationFunctionType.Sigmoid,
                        scale=1.0 / WR_SCALE,
                    )
                    yo = outp.tile([P, TB], F32, tag="yo")
                    nc.vector.tensor_tensor(
                        out=yo[:], in0=kv[m][:, chunks[h]], in1=rm[:],
                        op=mybir.AluOpType.mult,
                    )
                    nc.sync.dma_start(yout[m, :, chunks[h]], yo[:])

    nc.compile()
    return nc


def _routing(token_ids: np.ndarray):
    """Token -> (per-core global token list [E, CAP], per-core keep mask)."""
    tid = token_ids.reshape(N).astype(np.int64)
    eidx = (tid * HASH_PRIME) % E
    order = np.argsort(eidx, kind="stable")  # FIFO within expert
    counts = np.bincount(eidx, minlength=E)
    starts = np.zeros(E + 1, np.int64)
    np.cumsum(counts, out=starts[1:])

    token_lists = np.empty((E, CAP), np.int64)
    masks = np.zeros((E, CAP), np.float32)
    dropped = []
    fill_needed = []
    for e in range(E):
        grp = order[starts[e]:starts[e + 1]]
        nk = min(len(grp), CAP)
        token_lists[e, :nk] = grp[:nk]
        masks[e, :nk] = 1.0
        dropped.append(grp[CAP:])
        fill_needed.append(CAP - nk)
    dropped = (
        np.concatenate(dropped) if dropped else np.empty(0, np.int64)
    )
    pos = 0
    for e in range(E):
        need = fill_needed[e]
        if need:
            token_lists[e, CAP - need:] = dropped[pos:pos + need]
            pos += need
    assert pos == len(dropped)
    return token_lists, masks


def _tile_first(W, mt):
    """[C, M] -> [mt, P, CT*P] with w[m][p][k*P+q] = W[k*P+p, m*P+q]."""
    ct = W.shape[0] // P
    return np.ascontiguousarray(
        W.reshape(ct, P, mt, P).transpose(2, 1, 0, 3).reshape(mt, P, ct * P)
    ).astype(ml_dtypes.bfloat16)


def _tile_second(W, ct_out):
    """[K, M] -> [ct_out, P, KT*P] with w[m][p][k*P+q] = W[k*P+p, m*P+q]."""
    kt = W.shape[0] // P
    return np.ascontiguousarray(
        W.reshape(kt, P, ct_out, P).transpose(2, 1, 0, 3).reshape(ct_out, P, kt * P)
    ).astype(ml_dtypes.bfloat16)


def kernel(x, shift_state, token_ids, time_maa_k, time_maa_r, Wk, Wv, Wr, Wek, Wev):
    global _COMPILED
    if _COMPILED is None:
        _COMPILED = _build()
    nc = _COMPILED

    x = np.asarray(x, np.float32)
    shift_state = np.asarray(shift_state, np.float32)
    token_lists, masks = _routing(np.asarray(token_ids))

    # token shift on host (pure data movement + broadcast mix)
    xf = x.reshape(N, C)
    xprev_f = np.empty_like(xf)
    xprev_f[1:] = xf[:-1]
    xprev_f[np.arange(B) * T] = shift_state
    maak = np.asarray(time_maa_k, np.float32)
    maar = np.asarray(time_maa_r, np.float32)
    dx = xprev_f - xf
    xk_f = (xf + dx * maak).astype(ml_dtypes.bfloat16)
    xr_f = (xf + dx * maar).astype(ml_dtypes.float8_e4m3fn)

    wk_t = _tile_first(np.asarray(Wk, np.float32), MT_D)
    # Wr: [C, C] -> [m=CT, P, j=CT//2, i=2, q=P] fp8 DoubleRow layout with
    # wr8[m][p][j][i][q] = Wr[(2j+i)*P + p, m*P + q] * WR_SCALE
    wr_t = np.ascontiguousarray(
        (np.asarray(Wr, np.float32) * WR_SCALE)
        .reshape(CT // 2, 2, P, CT, P).transpose(3, 2, 0, 1, 4)
    ).astype(ml_dtypes.float8_e4m3fn)
    wv_t = _tile_second(np.asarray(Wv, np.float32), CT)
    Wek = np.asarray(Wek, np.float32)
    Wev = np.asarray(Wev, np.float32)

    def ctmajor(rows):  # [CAP, C] bf16 -> [CT, P, CAP]
        return np.ascontiguousarray(rows.T.reshape(CT, P, CAP))

    def ctpair(rows):  # [CAP, C] fp8 -> [CT//2, P, 2, CAP] (DoubleRow pairs)
        return np.ascontiguousarray(
            rows.T.reshape(CT // 2, 2, P, CAP).transpose(0, 2, 1, 3)
        )

    in_maps = []
    for e in range(E):
        L = token_lists[e]
        in_maps.append(dict(
            xk=ctmajor(xk_f[L]),
            xr=ctpair(xr_f[L]),
            maskd=np.ascontiguousarray(
                np.broadcast_to(masks[e], (P, CAP))
            ).astype(ml_dtypes.bfloat16),
            wk=wk_t,
            wv=wv_t,
            wr=wr_t,
            wek=_tile_first(Wek[e], MT_E),
            wev=_tile_second(Wev[e], CT),
        ))

    res = run_bass_kernel_spmd(
        nc, in_maps, core_ids=list(range(E)),
        trace=bool(os.environ.get("KERNEL_TRACE")),
    )
    global LAST_RESULTS
    LAST_RESULTS = res

    y = np.empty((N, C), np.float32)
    for e in range(E):
        y[token_lists[e]] = res.results[e]["y"].reshape(C, CAP).T
    return y.reshape(B, T, C)
